# revision 4
# baseline (speedup 1.0000x reference)
"""Trainium2 Bass kernel for nn_FHNet (batch[64,2048,784] @ W1.T -> FHN scan
-> *0.5 @ W2.T -> FHN scan), data-parallel over batch across 8 NeuronCores.

Per core (8 samples):
- mm1 on PE: K=784 in 7 chunks, batch pre-transposed host-side to
  [8, 784, 2048] so the contraction dim lands on partitions. Weights
  pre-scaled host-side so all constant folds (dt, 0.5, k/beta rescale,
  gamma shift) are free.
- FHN scans as per-step stock DVE ops on [100, 8] (scan1: feature on
  partitions, samples on free) / [80, 1] (scan2) column slices.
  Rescaled recurrence (Vt = k*V, Z = (k/beta)*(q - W)):
      Vt' = Vt*(A - Vt^2) + beta*Z ;  Z' = alpha*Z - Vt + p_t
  with p precomputed in bulk from the matmul outputs.
- mm2 on PE (K=100, single matmul per 512-token chunk).
- Output via strided DMA ([10, T] SBUF -> [T, 10] DRAM).

This container's walrus accepts at most ONE sync wait per engine
instruction; Tile emits more. `_split_multi_waits` hoists extras into
preceding same-engine EventSemaphore instructions (in-order execution
keeps semantics identical).
"""
import json
import sys
import numpy as np

sys.path.insert(0, "/opt/trn_rl_repo")

# ---------------- constants ----------------
DT = 0.04
A_CONST = float(1.0 + DT)
ALPHA = float(1.0 - DT * 0.08 * 0.8)
BETA = float(DT * DT * 0.08)
GAMMA = float(DT * DT * 0.08 * 0.7)
K_SC = float(np.sqrt(DT / 3.0))
KOB = float(K_SC / BETA)
C_BIAS = float(KOB * GAMMA / (1.0 - ALPHA))

B, T, D, N, M = 64, 2048, 784, 100, 10
NCORES = 8
BL = B // NCORES
KC = 7
KCH = D // KC          # 112
TCH = 512
NTC = T // TCH
TP = T + 1

_CACHE = {}


# ------------- walrus single-wait workaround -------------
def _split_multi_waits(bir_json_bytes: bytes) -> bytes:
    d = json.loads(bir_json_bytes)
    for fn in d.get("functions", []):
        for blk in fn.get("blocks", []):
            out = []
            for inst in blk.get("instructions", []):
                si = inst.get("sync_info")
                waits = (si or {}).get("on_wait") or []
                if len(waits) > 1:
                    for k, w in enumerate(waits[:-1]):
                        ev = {
                            "engine": inst["engine"],
                            "ins": [],
                            "outs": [],
                            "name": f"{inst['name']}_hw{k}",
                            "opcode": "EventSemaphore",
                            "sync_info": {"on_update": [], "on_wait": [w]},
                        }
                        if "debug" in inst:
                            ev["debug"] = inst["debug"]
                        out.append(ev)
                    si["on_wait"] = waits[-1:]
                out.append(inst)
            blk["instructions"] = out
    return json.dumps(d).encode()


def _install_bir_patch():
    import concourse.bass_utils as bu
    import concourse.bass2jax as b2j

    if getattr(bu, "_multiwait_patched", False):
        return
    orig = bu.compile_bir_kernel

    def patched(bir_json, tmpdir, neff_name="file.neff"):
        if isinstance(bir_json, str):
            bir_json = bir_json.encode()
        return orig(_split_multi_waits(bir_json), tmpdir, neff_name=neff_name)

    bu.compile_bir_kernel = patched
    bu._multiwait_patched = True
    b2j.compile_bir_kernel = patched


def _build_kernel():
    import concourse.bass as bass
    import concourse.tile as tile
    from concourse import mybir

    f32 = mybir.dt.float32
    AOp = mybir.AluOpType

    nc = bass.Bass()
    # register the bias constant for ScalarE add (activation bias const-AP)
    _cb = nc.alloc_sbuf_tensor("const-cbias", [128, 1], f32)
    nc.gpsimd.memset(_cb.ap(), -C_BIAS)
    nc.const_aps.aps[(f32, -C_BIAS)] = _cb.ap()
    nc.all_engine_barrier()

    bt_d = nc.declare_dram_parameter("batchT", [BL, D, T], f32, isOutput=False)
    w1_d = nc.declare_dram_parameter("W1T", [D, N], f32, isOutput=False)
    w2_d = nc.declare_dram_parameter("W2T", [N, M], f32, isOutput=False)
    out_d = nc.declare_dram_parameter("out", [BL, T, M], f32, isOutput=True)

    with tile.TileContext(nc) as tc:
        with (
            tc.tile_pool(name="const", bufs=1) as cpool,
            tc.tile_pool(name="bt", bufs=3) as btpool,
            tc.tile_pool(name="qs", bufs=3) as qspool,
            tc.tile_pool(name="ps1", bufs=4, space="PSUM") as ps1pool,
            tc.tile_pool(name="ps2", bufs=2, space="PSUM") as ps2pool,
            tc.tile_pool(name="big", bufs=1) as bigpool,
            tc.tile_pool(name="small", bufs=3) as spool,
            tc.tile_pool(name="state", bufs=2) as stpool,
        ):
            w1t = cpool.tile([KCH, KC * N], f32)
            for i in range(KC):
                nc.sync.dma_start(
                    w1t[:, i * N:(i + 1) * N], w1_d[i * KCH:(i + 1) * KCH, :]
                )
            w2t = cpool.tile([N, M], f32)
            nc.sync.dma_start(w2t[:], w2_d[:])

            p1 = bigpool.tile([N, BL * TP], f32)    # scan-1 p-stream, per sample
            v1 = bigpool.tile([N, BL * T], f32)     # scan-1 output (Vt1)
            q2 = bigpool.tile([BL * M, T], f32)     # q-hat-2, reused as Vt2
            p2 = bigpool.tile([BL * M, TP], f32)    # scan-2 p-stream

            p1_3 = p1[:].rearrange("p (b t) -> p b t", b=BL)
            v1_3 = v1[:].rearrange("p (b t) -> p b t", b=BL)

            # ---------------- mm1 + per-sample stream build ------------------
            for b in range(BL):
                qb = qspool.tile([N, T], f32, tag="qhat")
                for c in range(NTC):
                    ps = ps1pool.tile([N, TCH], f32)
                    for i in range(KC):
                        bt = btpool.tile([KCH, TCH], f32)
                        nc.sync.dma_start(
                            bt[:],
                            bt_d[b, i * KCH:(i + 1) * KCH,
                                 c * TCH:(c + 1) * TCH],
                        )
                        nc.tensor.matmul(
                            ps[:], lhsT=w1t[:, i * N:(i + 1) * N], rhs=bt[:],
                            start=(i == 0), stop=(i == KC - 1),
                        )
                    nc.scalar.add(qb[:, c * TCH:(c + 1) * TCH], ps[:], -C_BIAS)
                s = p1[:, b * TP:(b + 1) * TP]
                nc.vector.tensor_scalar(
                    s[:, 0:1], qb[:, 0:1], C_BIAS, None, AOp.add)
                nc.vector.scalar_tensor_tensor(
                    s[:, 1:T], qb[:, 0:T - 1], -ALPHA, qb[:, 1:T],
                    AOp.mult, AOp.add)
                nc.gpsimd.memset(s[:, T:T + 1], 0.0)

            # ---------------- scan 1 (all samples per instruction) ----------
            # state cols: V_t lives in v1[:, :, t]; Z in ping-pong tiles.
            nc.vector.memset(v1_3[:, :, 0], 0.0)
            za = stpool.tile([N, BL], f32, tag="za")
            zb = stpool.tile([N, BL], f32, tag="zb")
            # Z_0 = qhat_0 + c  (= p-stream col 0 + ... col0 holds qhat0 + c)
            nc.vector.tensor_copy(za[:], p1_3[:, :, 0])
            SQ = mybir.ActivationFunctionType.Square
            for t in range(T - 1):
                zs, zn = (za, zb) if t % 2 == 0 else (zb, za)
                vt = v1_3[:, :, t]
                u = spool.tile([N, BL], f32, tag="u")
                nc.scalar.activation(u[:], vt, SQ)        # u = V^2 (ScalarE)
                m = spool.tile([N, BL], f32, tag="m")
                # m' = (u - A) * V  == -(A - u)*V
                nc.vector.scalar_tensor_tensor(
                    m[:], u[:], A_CONST, vt, AOp.subtract, AOp.mult)
                # V' = beta*Z - m'
                nc.vector.scalar_tensor_tensor(
                    v1_3[:, :, t + 1], zs[:], BETA, m[:], AOp.mult,
                    AOp.subtract)
                t2 = spool.tile([N, BL], f32, tag="t2")
                nc.vector.scalar_tensor_tensor(
                    t2[:], zs[:], ALPHA, vt, AOp.mult, AOp.subtract)
                nc.vector.tensor_tensor(zn[:], t2[:], p1_3[:, :, t + 1],
                                        AOp.add)

            # ---------------- mm2 ------------------------------------------
            for b in range(BL):
                for c in range(NTC):
                    ps2 = ps2pool.tile([M, TCH], f32)
                    nc.tensor.matmul(
                        ps2[:], lhsT=w2t[:],
                        rhs=v1[:, b * T + c * TCH: b * T + (c + 1) * TCH],
                        start=True, stop=True)
                    st2 = spool.tile([M, TCH], f32, tag="q2st")
                    nc.scalar.add(st2[:], ps2[:], -C_BIAS)
                    nc.sync.dma_start(
                        q2[b * M:(b + 1) * M, c * TCH:(c + 1) * TCH], st2[:])

            # ---------------- scan 2 ([80, 1] slices) -----------------------
            nc.vector.tensor_scalar(
                p2[:, 0:1], q2[:, 0:1], C_BIAS, None, AOp.add)
            nc.vector.scalar_tensor_tensor(
                p2[:, 1:T], q2[:, 0:T - 1], -ALPHA, q2[:, 1:T],
                AOp.mult, AOp.add)
            nc.gpsimd.memset(p2[:, T:T + 1], 0.0)

            P2 = BL * M
            za2 = stpool.tile([P2, 1], f32, tag="za2")
            zb2 = stpool.tile([P2, 1], f32, tag="zb2")
            nc.vector.tensor_copy(za2[:], p2[:, 0:1])
            nc.vector.memset(q2[:, 0:1], 0.0)   # V2 col 0 (q2 reused as V2)
            for t in range(T - 1):
                zs, zn = (za2, zb2) if t % 2 == 0 else (zb2, za2)
                vt = q2[:, t:t + 1]
                u = spool.tile([P2, 1], f32, tag="u2")
                nc.vector.tensor_tensor(u[:], vt, vt, AOp.mult)
                r = spool.tile([P2, 1], f32, tag="r2")
                nc.vector.tensor_scalar(r[:], u[:], -1.0, A_CONST,
                                        AOp.mult, AOp.add)
                m = spool.tile([P2, 1], f32, tag="m2")
                nc.vector.tensor_tensor(m[:], vt, r[:], AOp.mult)
                nc.vector.scalar_tensor_tensor(
                    q2[:, t + 1:t + 2], zs[:], BETA, m[:], AOp.mult, AOp.add)
                t2 = spool.tile([P2, 1], f32, tag="t22")
                nc.vector.scalar_tensor_tensor(
                    t2[:], zs[:], ALPHA, vt, AOp.mult, AOp.subtract)
                nc.vector.tensor_tensor(zn[:], t2[:], p2[:, t + 1:t + 2],
                                        AOp.add)

            # unscale into p2 (dead) and DMA out
            nc.vector.tensor_scalar(p2[:, 0:T], q2[:], 1.0 / K_SC, None,
                                    AOp.mult)
            for b in range(BL):
                nc.sync.dma_start(
                    out_d[b].rearrange("t m -> m t"),
                    p2[b * M:(b + 1) * M, 0:T])

    return nc


def kernel(batch, W1, W2):
    _install_bir_patch()
    from concourse.bass_utils import run_bass_kernel_spmd

    if "nc" not in _CACHE:
        _CACHE["nc"] = _build_kernel()
    nc = _CACHE["nc"]

    batch = np.asarray(batch, dtype=np.float32)
    W1 = np.asarray(W1, dtype=np.float32)
    W2 = np.asarray(W2, dtype=np.float32)

    w1t = np.ascontiguousarray((KOB * DT * W1).T.astype(np.float32))
    w2t = np.ascontiguousarray(((DT * 0.5 / BETA) * W2).T.astype(np.float32))

    in_maps = []
    for cidx in range(NCORES):
        sl = batch[cidx * BL:(cidx + 1) * BL]
        btT = np.ascontiguousarray(sl.transpose(0, 2, 1))
        in_maps.append({"batchT": btT, "W1T": w1t, "W2T": w2t})

    res = run_bass_kernel_spmd(nc, in_maps, list(range(NCORES)))
    out = np.concatenate([res.results[i]["out"] for i in range(NCORES)], axis=0)
    return out.astype(np.float32)
